# revision 14
# baseline (speedup 1.0000x reference)
"""Multi-head attention kernel for 8 Trainium2 NeuronCores (v5).

Problem: B=2, S=2048, D=1024, H=16 heads (Dh=64).
    qh = split(q @ wq.T + bq); kh, vh likewise
    out = concat_h(softmax(qh kh^T / 8) vh) @ wo.T + bo

Sharding: core c = 4*b + j handles batch b and head group j (4 heads,
channels [256j, 256j+256)).  Each core computes its 4 heads' attention and
a partial output projection; the host sums the 4 partials per batch and
adds the constant bv @ wo.T + bo vector.

v5 structure (changes vs v4, driven by the 222.5us trace):
  - Prologue: wq/wk are staged cc-major ([2,128,8,128]) so the DMA
    critical prefix is wk0,xk0,bk,wq0,xq0,bq (2.5MB); prologue computes
    only k_group(0,0) + q_group(0,0); first exp ~7us earlier.  Extra
    warm-up matmuls bridge the DMA wait so HAM stays at 8/8 (v4 idled
    11.3->16.1us and re-throttled, running all projections at 1.2GHz).
  - V's ones-column (softmax denominator) comes from one memset of the
    whole V tile instead of 4 strided DMAs (v4 burned 12us of sync-queue
    dispatch on 2-byte-element DMA descriptors).
  - Units run hp-major: (0,0),(1,0),(2,0),(3,0),(0,1)...(3,1), so the
    cc=1 projections spread into the second half of the schedule.  Unit
    (0,0) is structurally PE-bound (all 16 v_groups must land inside it
    - PV(T) consumes V[T] there); its item slots are packed with exact
    deadline tracking and its PV pend drains 2/kt from kt12 so the
    boundary bubble is ~1.7us instead of ~4us.
  - 11 of 128 exp tiles are computed on the DVE instead of ACT
    (Schraudolph: i16 = round(s*23.083 + 16250.24) bit-cast as bf16;
    HW-validated rounds-to-nearest, max rel err 3.3%).  ACT is the
    binding engine in units 1-7 (1146ns per [128,1024] exp, 147us total
    for all 128); each offloaded tile buys 1.15us.  Offload only in
    ACT-bound units (never unit 0).
  - Tail: outproj(2,2/3) runs after the last unit's drain concurrently
    with a 128-token-chunked norm(3,1); outproj(3,t) follows chunk t;
    tail copies split ACT+DVE; out partials are bf16 (halves the final
    DMA drain; host sums in f32).

Known-good v4: rel err 6.64e-3, 221.8-226.4us.
"""

import numpy as np
import ml_dtypes
import concourse.bass as bass
import concourse.tile as tile
import concourse.mybir as mybir
from concourse import bacc, bass_utils

B, S, D, H = 2, 2048, 1024, 16
DH = 64
HL = 4            # heads per core
CL = HL * DH      # local channels = 256
N_CORES = 8

f32 = mybir.dt.float32
bf16 = mybir.dt.bfloat16
i16 = mybir.dt.int16
AF = mybir.ActivationFunctionType
ALU = mybir.AluOpType
BF = ml_dtypes.bfloat16

TB = 4            # token blocks for projections (512 tokens each)
TBS = S // TB     # 512
QB = 4            # query blocks for attention (512 queries each)
QBS = S // QB     # 512
KT_N = S // 128   # 16 key tiles

# Schraudolph exp for the DVE-offloaded tiles: e ~= bf16_bits(round(A*s + B))
# A = 128*log2(e)*0.125 (the 1/8 score scale folded in), B = 128*(127+d),
# d = -0.045 balances the periodic relative error to ~+-3%.
EXP_A = 128.0 * 1.4426950408889634 * 0.125
EXP_B = 128.0 * (127.0 - 0.045)

# unit order: hp-major so cc=1 projections defer to the second half
UNITS = [(0, 0), (1, 0), (2, 0), (3, 0), (0, 1), (1, 1), (2, 1), (3, 1)]
# kt indices whose exp runs on DVE, per unit index (never unit 0: PE-bound)
DVE_EXP = {1: (6, 11), 2: (6, 11), 3: (6, 11), 4: (6, 11),
           5: (8,), 6: (8,), 7: (8,)}


def build():
    nc = bacc.Bacc("TRN2", debug=False, num_devices=N_CORES)
    qt4 = nc.dram_tensor("qt4", [TB, 128, 8, TBS], bf16, kind="ExternalInput").ap()
    kt4 = nc.dram_tensor("kt4", [TB, 128, 8, TBS], bf16, kind="ExternalInput").ap()
    vt4 = nc.dram_tensor("vt4", [TB, 128, 8, TBS], bf16, kind="ExternalInput").ap()
    wqT = nc.dram_tensor("wqT", [2, 128, 8, 128], bf16, kind="ExternalInput").ap()
    wkT = nc.dram_tensor("wkT", [2, 128, 8, 128], bf16, kind="ExternalInput").ap()
    wvT = nc.dram_tensor("wvT", [128, 8, CL], bf16, kind="ExternalInput").ap()
    woT = nc.dram_tensor("woT", [128, 2, D], bf16, kind="ExternalInput").ap()
    bq = nc.dram_tensor("bq", [128, 2], f32, kind="ExternalInput").ap()
    bk = nc.dram_tensor("bk", [128, 2], f32, kind="ExternalInput").ap()
    out = nc.dram_tensor("out", [S, D], bf16, kind="ExternalOutput").ap()

    with tile.TileContext(nc) as tc:
        with (
            tc.tile_pool(name="wp", bufs=1) as wp,
            tc.tile_pool(name="xp", bufs=12) as xp,
            tc.tile_pool(name="qkv", bufs=1) as qkv,
            tc.tile_pool(name="cp", bufs=1) as cp,
            tc.tile_pool(name="ep", bufs=7) as ep,
            tc.tile_pool(name="rp", bufs=2) as rp,
            tc.tile_pool(name="op", bufs=2) as op,
            tc.tile_pool(name="pp", bufs=2, space="PSUM") as pp,
            tc.tile_pool(name="sp", bufs=2, space="PSUM") as sp,
            tc.tile_pool(name="cps", bufs=1, space="PSUM") as cps,
        ):
            # ---- constants first (no DMA deps) so warm-up can start now ----
            ones_sb = wp.tile([128, 64], bf16)
            nc.vector.memset(ones_sb, 1.0)
            warm_rhs = wp.tile([128, 512], bf16)
            nc.vector.memset(warm_rhs, 0.0)

            # V[tok, kt, head-of-4, 65]: col 64 of each head group stays at the
            # memset 1.0 -> PV row 64 is the softmax denominator.
            V = qkv.tile([128, KT_N, 4, 65], bf16)
            nc.vector.memset(V, 1.0)

            # warm-up matmuls: occupy the PE through the input-DMA wait so the
            # HAM clock gate reaches and KEEPS 8/8 until k_group(0,0) starts.
            for i in range(14):
                wps = pp.tile([64, 512], f32, tag="pp", name="wps")
                nc.tensor.matmul(wps, ones_sb[:, 0:64], warm_rhs)

            # ---- weights; DMA order = need order -------------------------
            wk_sb = wp.tile([128, 2, 8, 128], bf16)
            wq_sb = wp.tile([128, 2, 8, 128], bf16)
            wv_sb = wp.tile([128, 8, CL], bf16)
            wo_sb = wp.tile([128, 2, D], bf16)
            bq_sb = wp.tile([128, 2], f32)
            bk_sb = wp.tile([128, 2], f32)

            xk_t = [None] * TB
            xv_t = [None] * TB
            xq_t = [None] * QB

            def dma_x(kind, idx):
                t = xp.tile([128, 8, TBS], bf16, tag="x", name=f"x{kind}{idx}")
                src = {"k": kt4, "v": vt4, "q": qt4}[kind]
                nc.sync.dma_start(t, src[idx])
                return t

            # The 16 DMA queues drain in dispatch order at ~0.4GB/ms marginal,
            # ~2.4us per MB; dispatch order IS the arrival order, so it must
            # exactly match the compute need order (v5's xq1-at-slot-2 miss
            # stalled the PE FIFO 5.7us).  Everything is dispatched upfront
            # (xp has 12 bufs - no reuse, no item-DMA deadlocks).
            nc.sync.dma_start(wk_sb[:, 0], wkT[0])
            nc.sync.dma_start(bk_sb, bk)
            xk_t[0] = dma_x("k", 0)
            nc.sync.dma_start(wq_sb[:, 0], wqT[0])
            xq_t[0] = dma_x("q", 0)
            nc.sync.dma_start(bq_sb, bq)
            nc.sync.dma_start(wv_sb, wvT)
            xk_t[1] = dma_x("k", 1)
            xv_t[0] = dma_x("v", 0)
            xk_t[2] = dma_x("k", 2)
            xv_t[1] = dma_x("v", 1)
            xk_t[3] = dma_x("k", 3)
            xv_t[2] = dma_x("v", 2)
            xv_t[3] = dma_x("v", 3)
            xq_t[1] = dma_x("q", 1)
            nc.sync.dma_start(wk_sb[:, 1], wkT[1])
            xq_t[2] = dma_x("q", 2)
            nc.sync.dma_start(wq_sb[:, 1], wqT[1])
            xq_t[3] = dma_x("q", 3)
            nc.sync.dma_start(wo_sb, woT)

            # ---- activations in SBUF ----
            QT = qkv.tile([128, 2, S], bf16)   # [chan, tok]
            KT = qkv.tile([128, 2, S], bf16)

            # ---- projection group emitters ----
            def k_group(tb, cc):
                t0 = tb * TBS
                ps = pp.tile([128, TBS], f32, tag="pp", name="ps_k")
                for c in range(8):
                    nc.tensor.matmul(ps, wk_sb[:, cc, c], xk_t[tb][:, c],
                                     start=(c == 0), stop=(c == 7))
                nc.vector.tensor_scalar_add(KT[:, cc, t0:t0 + TBS], ps,
                                            bk_sb[:, cc:cc + 1])

            def q_group(qb, cc):
                t0 = qb * TBS
                ps = pp.tile([128, TBS], f32, tag="pp", name="ps_q")
                for c in range(8):
                    nc.tensor.matmul(ps, wq_sb[:, cc, c], xq_t[qb][:, c],
                                     start=(c == 0), stop=(c == 7))
                nc.vector.tensor_scalar_add(QT[:, cc, t0:t0 + TBS], ps,
                                            bq_sb[:, cc:cc + 1])

            def v_group(tb, tt):
                ps = pp.tile([128, CL], f32, tag="pp", name="ps_v")
                for c in range(8):
                    nc.tensor.matmul(ps, xv_t[tb][:, c, tt * 128:(tt + 1) * 128],
                                     wv_sb[:, c], start=(c == 0), stop=(c == 7))
                T = tb * 4 + tt
                nc.vector.tensor_copy(
                    V[:, T, :, 0:64],
                    ps.rearrange("p (g c) -> p g c", c=64))

            # ---- prologue projections: just enough for unit (0,0) kt0 ----
            k_group(0, 0)
            q_group(0, 0)

            # ---- attention ----
            C = cp.tile([128, 2, S], bf16)   # C^T [cat-chan, tok]

            def make_norm(qb, hp, c):
                q0 = qb * QBS

                def norm():
                    # c is [65, 2, QBS]; row 64 = denominators for both heads
                    s_ab = rp.tile([65, 2, QBS], bf16, tag="sab", name="s_ab")
                    nc.vector.tensor_copy(s_ab[64:65], c[64:65])
                    b_ps = pp.tile([64, QBS], f32, tag="pp", name="b_ps")
                    nc.tensor.matmul(b_ps, ones_sb[64:65, :], s_ab[64:65, 0])
                    r_a = rp.tile([64, QBS], f32, tag="ra", name="r_a")
                    nc.vector.reciprocal_approx_fast(r_a, b_ps)
                    b_ps2 = pp.tile([64, QBS], f32, tag="pp", name="b_ps2")
                    nc.tensor.matmul(b_ps2, ones_sb[64:65, :], s_ab[64:65, 1])
                    r_b = rp.tile([64, QBS], f32, tag="rb", name="r_b")
                    nc.vector.reciprocal_approx_fast(r_b, b_ps2)
                    nc.vector.tensor_mul(C[0:64, hp, q0:q0 + QBS],
                                         c[0:64, 0], r_a)
                    nc.vector.tensor_mul(C[64:128, hp, q0:q0 + QBS],
                                         c[0:64, 1], r_b)
                return norm

            def make_outproj(qb, tt, act_copy=False):
                def outproj():
                    tg = qb * QBS + tt * 128
                    o = op.tile([128, D], bf16, tag="o", name="o")
                    ps0 = pp.tile([128, 512], f32, tag="pp", name="ps0")
                    ps1 = pp.tile([128, 512], f32, tag="pp", name="ps1")
                    for cc in range(2):
                        nc.tensor.matmul(ps0, C[:, cc, tg:tg + 128],
                                         wo_sb[:, cc, 0:512],
                                         start=(cc == 0), stop=(cc == 1))
                        nc.tensor.matmul(ps1, C[:, cc, tg:tg + 128],
                                         wo_sb[:, cc, 512:1024],
                                         start=(cc == 0), stop=(cc == 1))
                    if act_copy:
                        nc.scalar.copy(o[:, 0:512], ps0)
                    else:
                        nc.vector.tensor_copy(o[:, 0:512], ps0)
                    nc.vector.tensor_copy(o[:, 512:1024], ps1)
                    nc.sync.dma_start(out[tg:tg + 128, :], o)
                return outproj

            def seq(*fns):
                def run():
                    for f in fns:
                        f()
                return run

            def mk(fn, *args):
                return lambda: fn(*args)

            # ---- per-unit interleave schedules ----
            # unit 0 carries all v_groups (V[T] is consumed by its PV(T)) plus
            # the cc=0 k-projections; slots are deadline-exact (see docstring).
            items = {u: [] for u in range(len(UNITS))}
            # pops: PV(j-4) at kt j (j=4..11); kt12 pops PV(8..10); kt13-15
            # pop PV(11..13); leftover items precede PV(14),PV(15).  Every
            # v_group(T) below is emitted at/before the kt whose pop reads it;
            # the DMA arrival order above matches these slots.  None = empty
            # slot (in units 1-7 heavy PE items are spaced ~3 kts apart so the
            # ACT stream, which can only run ~1 exp ahead via the 2 score
            # buffers, recovers between them).
            # scores(kt) is emitted at step kt-2 BEFORE that step's item, so
            # k_group(tb,0) must sit at step <= 4*tb-3.
            items[0] = [
                mk(k_group, 1, 0),                                # kt1 <=kt1
                mk(v_group, 0, 0),                                # kt2  V0<=kt4
                mk(v_group, 0, 1),                                # kt3  V1<=kt5
                mk(v_group, 0, 2),                                # kt4  V2<=kt6
                mk(k_group, 2, 0),                                # kt5 <=kt5
                mk(v_group, 0, 3),                                # kt6  V3<=kt7
                mk(v_group, 1, 0),                                # kt7  V4<=kt8
                mk(v_group, 1, 1),                                # kt8  V5<=kt9
                seq(mk(k_group, 3, 0), mk(v_group, 1, 2)),        # kt9  V6<=kt10
                mk(v_group, 1, 3),                                # kt10 V7<=kt11
                mk(v_group, 2, 0),                                # kt11 V8<=kt12
                seq(mk(v_group, 2, 1), mk(v_group, 2, 2)),        # kt12 V9,V10
                mk(v_group, 2, 3),                                # kt13 V11<=kt13
                mk(v_group, 3, 0),                                # kt14 V12<=kt14
                seq(mk(q_group, 1, 0), mk(v_group, 3, 1)),        # kt15 V13<=kt15
                mk(v_group, 3, 2),                                # leftover V14
                mk(v_group, 3, 3),                                # leftover V15
            ]
            items[1] = [None, mk(k_group, 0, 1), None, None,
                        mk(q_group, 2, 0)]
            items[2] = [None, mk(q_group, 3, 0), None, None,
                        mk(k_group, 1, 1)]
            items[3] = [None, mk(q_group, 0, 1), None, None,
                        mk(k_group, 2, 1)]
            items[4] = [None, mk(q_group, 1, 1), None, None,
                        mk(k_group, 3, 1)]
            # outprojs start once both norms of a qb are done (norm(qb,1) of
            # unit 4+i is the first item of unit 5+i)
            items[5] = [make_outproj(0, 0), None, make_outproj(0, 1), None,
                        mk(q_group, 2, 1)]
            items[6] = [make_outproj(0, 2), None, make_outproj(0, 3), None,
                        make_outproj(1, 0), None, mk(q_group, 3, 1)]
            items[7] = [make_outproj(1, 1), None, make_outproj(1, 2), None,
                        make_outproj(1, 3), None, make_outproj(2, 0), None,
                        make_outproj(2, 1)]

            def pv(kt, e, c, hp):
                # C^T accumulation; row 64 = softmax denominators
                nc.tensor.matmul(c[:, 0], V[:, kt, 2 * hp],
                                 e[:, 0:QBS], start=(kt == 0),
                                 stop=(kt == KT_N - 1))
                nc.tensor.matmul(c[:, 1], V[:, kt, 2 * hp + 1],
                                 e[:, QBS:2 * QBS], start=(kt == 0),
                                 stop=(kt == KT_N - 1))

            last_es = []

            def attention_unit(ui, work, finish_prev=None, keep_es=False):
                # Software-pipelined: scores(kt+2) is emitted right after
                # exp(kt) and BEFORE the kt's work item, so the ACT stream has
                # a 2-exp runway (the 2 score buffers allow it; v5.1's 1-ahead
                # emission made every work item and every DVE-offloaded exp an
                # ACT bubble).  The previous unit's PV drain is emitted after
                # this unit's first scores+exp (finish_prev) for the same
                # reason.
                qb, hp = UNITS[ui]
                q0 = qb * QBS
                c = cps.tile([65, 2, QBS], f32, tag="c", name="c")
                dve_kts = DVE_EXP.get(ui, ())

                def scores(kt):
                    k0 = kt * 128
                    s_ps = sp.tile([128, 2 * QBS], f32, tag="s")
                    nc.tensor.matmul(s_ps[:, 0:QBS],
                                     KT[0:64, hp, k0:k0 + 128],
                                     QT[0:64, hp, q0:q0 + QBS])
                    nc.tensor.matmul(s_ps[:, QBS:2 * QBS],
                                     KT[64:128, hp, k0:k0 + 128],
                                     QT[64:128, hp, q0:q0 + QBS])
                    return s_ps

                s_tile = {}
                pend = []
                it = 0
                for kt in range(KT_N):
                    if kt == 0:
                        s_tile[0] = scores(0)
                        s_tile[1] = scores(1)
                    e = ep.tile([128, 2 * QBS], bf16, tag="e")
                    if kt in dve_kts:
                        nc.vector.tensor_scalar(e.bitcast(i16), s_tile.pop(kt),
                                                EXP_A, EXP_B, ALU.mult, ALU.add)
                    else:
                        nc.scalar.activation(e, s_tile.pop(kt), AF.Exp,
                                             scale=0.125)
                    if kt == 0 and finish_prev is not None:
                        finish_prev()
                    if kt + 2 < KT_N:
                        s_tile[kt + 2] = scores(kt + 2)
                    if keep_es and kt >= 13:
                        last_es.append(e)
                    pend.append((kt, e))
                    if kt >= 1 and it < len(work):
                        if work[it] is not None:
                            work[it]()
                        it += 1
                    # drain 2/kt from kt12 so the end-of-unit PV tail is short
                    npend = 4 if kt < 12 else 2
                    while len(pend) > npend:
                        pv(*pend.pop(0), c, hp)

                def finisher():
                    j = it
                    while j < len(work):
                        if work[j] is not None:
                            work[j]()
                        j += 1
                    for item in pend:
                        pv(*item, c, hp)
                return c, finisher

            prev_c = {}
            fin = None
            for ui in range(len(UNITS)):
                work = list(items[ui])
                if ui >= 1:
                    pu = UNITS[ui - 1]
                    work.insert(0, make_norm(pu[0], pu[1], prev_c[ui - 1]))
                prev_c[ui], fin = attention_unit(
                    ui, work, finish_prev=fin,
                    keep_es=(ui == len(UNITS) - 1))
            fin()

            # ---- tail: outproj(2,2..3) runs during the chunked norm(3,1);
            # outproj(3,t) follows chunk t; copies split ACT+DVE ------------
            lc = prev_c[len(UNITS) - 1]
            make_outproj(2, 2, act_copy=True)()
            make_outproj(2, 3, act_copy=True)()
            q0 = 3 * QBS
            for t in range(4):
                g0 = t * 128
                s_ab = rp.tile([65, 2, 128], bf16, tag="sabt", name="s_abt")
                nc.vector.tensor_copy(s_ab[64:65], lc[64:65, :, g0:g0 + 128])
                b_ps = pp.tile([64, 128], f32, tag="pp", name="tb_ps")
                nc.tensor.matmul(b_ps, ones_sb[64:65, :], s_ab[64:65, 0])
                r_a = rp.tile([64, 128], f32, tag="rat", name="r_at")
                nc.vector.reciprocal_approx_fast(r_a, b_ps)
                b_ps2 = pp.tile([64, 128], f32, tag="pp", name="tb_ps2")
                nc.tensor.matmul(b_ps2, ones_sb[64:65, :], s_ab[64:65, 1])
                r_b = rp.tile([64, 128], f32, tag="rbt", name="r_bt")
                nc.vector.reciprocal_approx_fast(r_b, b_ps2)
                nc.vector.tensor_mul(C[0:64, 1, q0 + g0:q0 + g0 + 128],
                                     lc[0:64, 0, g0:g0 + 128], r_a)
                nc.vector.tensor_mul(C[64:128, 1, q0 + g0:q0 + g0 + 128],
                                     lc[0:64, 1, g0:g0 + 128], r_b)
                make_outproj(3, t, act_copy=True)()

            # HAM keep-warm junk through the final exp drain (reads the last
            # unit's e tiles, whose ep buffers are never rotated again)
            for e in last_es:
                for h in range(2):
                    wps = pp.tile([64, 512], f32, tag="pp", name="wps_t")
                    nc.tensor.matmul(wps, ones_sb[:, 0:64], e[:, h * 512:h * 512 + 512])

    nc.compile()
    return nc


_CACHE = {}


def _get_nc():
    if "nc" not in _CACHE:
        _CACHE["nc"] = build()
    return _CACHE["nc"]


def make_in_maps(q, k, v, wq, bq, wk, bk, wv, bv, wo, bo):
    def tile4(x):
        # [S, D] activation -> [TB, 128, 8, TBS]; [tb,p,c,t] = x[tb*512+t, c*128+p]
        xT = np.ascontiguousarray(np.asarray(x).T).astype(BF)  # [D, S]
        return np.ascontiguousarray(
            xT.reshape(8, 128, TB, TBS).transpose(2, 1, 0, 3))

    def wtile_cc(w, sl):
        # [D_out slice, D_in] -> [2, 128, 8, 128]; [cc,p,c,j] = w.T[c*128+p, cc*128+j]
        wT = np.ascontiguousarray(np.asarray(w)[sl, :].T).astype(BF)  # [D, CL]
        t = wT.reshape(8, 128, 2, 128).transpose(2, 1, 0, 3)
        return np.ascontiguousarray(t)

    def wvtile(w, sl):
        wT = np.ascontiguousarray(np.asarray(w)[sl, :].T).astype(BF)  # [D, CL]
        return np.ascontiguousarray(wT.reshape(8, 128, CL).transpose(1, 0, 2))

    def wotile(wo_, sl):
        woTc = np.ascontiguousarray(np.asarray(wo_)[:, sl].T).astype(BF)  # [CL, D]
        return np.ascontiguousarray(woTc.reshape(2, 128, D).transpose(1, 0, 2))

    def btile(b, sl):
        return np.ascontiguousarray(
            np.asarray(b)[sl].astype(np.float32).reshape(2, 128).T)

    xt = {}
    for b in range(B):
        xt[("q", b)] = tile4(q[b])
        xt[("k", b)] = tile4(k[b])
        xt[("v", b)] = tile4(v[b])
    in_maps = []
    for core in range(N_CORES):
        b, j = divmod(core, N_CORES // B)
        sl = slice(CL * j, CL * (j + 1))
        in_maps.append({
            "qt4": xt[("q", b)],
            "kt4": xt[("k", b)],
            "vt4": xt[("v", b)],
            "wqT": wtile_cc(wq, sl),
            "wkT": wtile_cc(wk, sl),
            "wvT": wvtile(wv, sl),
            "woT": wotile(wo, sl),
            "bq": btile(bq, sl),
            "bk": btile(bk, sl),
        })
    return in_maps


def combine(results, bv, wo, bo):
    GP = N_CORES // B
    const = (np.asarray(bv, dtype=np.float64) @ np.asarray(wo, dtype=np.float64).T
             + np.asarray(bo, dtype=np.float64)).astype(np.float32)
    out = np.empty((B, S, D), dtype=np.float32)
    for b in range(B):
        acc = results[b * GP]["out"].astype(np.float32)
        for j in range(1, GP):
            acc = acc + results[b * GP + j]["out"].astype(np.float32)
        out[b] = acc + const[None, :]
    return out


def kernel(q, k, v, wq, bq, wk, bk, wv, bv, wo, bo):
    nc = _get_nc()
    in_maps = make_in_maps(q, k, v, wq, bq, wk, bk, wv, bv, wo, bo)
    res = bass_utils.run_bass_kernel_spmd(nc, in_maps, core_ids=list(range(N_CORES)))
    return combine(res.results, bv, wo, bo)


# revision 18
# speedup vs baseline: 1.0190x; 1.0190x over previous
"""Multi-head attention kernel for 8 Trainium2 NeuronCores (v5).

Problem: B=2, S=2048, D=1024, H=16 heads (Dh=64).
    qh = split(q @ wq.T + bq); kh, vh likewise
    out = concat_h(softmax(qh kh^T / 8) vh) @ wo.T + bo

Sharding: core c = 4*b + j handles batch b and head group j (4 heads,
channels [256j, 256j+256)).  Each core computes its 4 heads' attention and
a partial output projection; the host sums the 4 partials per batch and
adds the constant bv @ wo.T + bo vector.

v5 structure (changes vs v4, driven by the 222.5us trace):
  - Prologue: wq/wk are staged cc-major ([2,128,8,128]) so the DMA
    critical prefix is wk0,xk0,bk,wq0,xq0,bq (2.5MB); prologue computes
    only k_group(0,0) + q_group(0,0); first exp ~7us earlier.  Extra
    warm-up matmuls bridge the DMA wait so HAM stays at 8/8 (v4 idled
    11.3->16.1us and re-throttled, running all projections at 1.2GHz).
  - V's ones-column (softmax denominator) comes from one memset of the
    whole V tile instead of 4 strided DMAs (v4 burned 12us of sync-queue
    dispatch on 2-byte-element DMA descriptors).
  - Units run hp-major: (0,0),(1,0),(2,0),(3,0),(0,1)...(3,1), so the
    cc=1 projections spread into the second half of the schedule.  Unit
    (0,0) is structurally PE-bound (all 16 v_groups must land inside it
    - PV(T) consumes V[T] there); its item slots are packed with exact
    deadline tracking and its PV pend drains 2/kt from kt12 so the
    boundary bubble is ~1.7us instead of ~4us.
  - 11 of 128 exp tiles are computed on the DVE instead of ACT
    (Schraudolph: i16 = round(s*23.083 + 16250.24) bit-cast as bf16;
    HW-validated rounds-to-nearest, max rel err 3.3%).  ACT is the
    binding engine in units 1-7 (1146ns per [128,1024] exp, 147us total
    for all 128); each offloaded tile buys 1.15us.  Offload only in
    ACT-bound units (never unit 0).
  - Tail: outproj(2,2/3) runs after the last unit's drain concurrently
    with a 128-token-chunked norm(3,1); outproj(3,t) follows chunk t;
    tail copies split ACT+DVE; out partials are bf16 (halves the final
    DMA drain; host sums in f32).

Known-good v4: rel err 6.64e-3, 221.8-226.4us.
"""

import numpy as np
import ml_dtypes
import concourse.bass as bass
import concourse.tile as tile
import concourse.mybir as mybir
from concourse import bacc, bass_utils

B, S, D, H = 2, 2048, 1024, 16
DH = 64
HL = 4            # heads per core
CL = HL * DH      # local channels = 256
N_CORES = 8

f32 = mybir.dt.float32
bf16 = mybir.dt.bfloat16
i16 = mybir.dt.int16
AF = mybir.ActivationFunctionType
ALU = mybir.AluOpType
BF = ml_dtypes.bfloat16

TB = 4            # token blocks for projections (512 tokens each)
TBS = S // TB     # 512
QB = 4            # query blocks for attention (512 queries each)
QBS = S // QB     # 512
KT_N = S // 128   # 16 key tiles

# Schraudolph exp for the DVE-offloaded tiles: e ~= bf16_bits(round(A*s + B))
# A = 128*log2(e)*0.125 (the 1/8 score scale folded in), B = 128*(127+d),
# d = -0.045 balances the periodic relative error to ~+-3%.
EXP_A = 128.0 * 1.4426950408889634 * 0.125
EXP_B = 128.0 * (127.0 - 0.045)

# unit order: hp-major so cc=1 projections defer to the second half
UNITS = [(0, 0), (1, 0), (2, 0), (3, 0), (0, 1), (1, 1), (2, 1), (3, 1)]
# kt indices whose exp runs on DVE, per unit index (never unit 0: PE-bound)
DVE_EXP = {1: (6, 11), 2: (6, 11), 3: (6, 11), 4: (6, 11),
           5: (8,), 6: (8,), 7: (8,)}


def build():
    nc = bacc.Bacc("TRN2", debug=False, num_devices=N_CORES)
    qt4 = nc.dram_tensor("qt4", [TB, 128, 8, TBS], bf16, kind="ExternalInput").ap()
    kt4 = nc.dram_tensor("kt4", [TB, 128, 8, TBS], bf16, kind="ExternalInput").ap()
    vt4 = nc.dram_tensor("vt4", [TB, 128, 8, TBS], bf16, kind="ExternalInput").ap()
    wqT = nc.dram_tensor("wqT", [2, 128, 8, 128], bf16, kind="ExternalInput").ap()
    wkT = nc.dram_tensor("wkT", [2, 128, 8, 128], bf16, kind="ExternalInput").ap()
    wvT = nc.dram_tensor("wvT", [128, 8, CL], bf16, kind="ExternalInput").ap()
    woT = nc.dram_tensor("woT", [128, 2, D], bf16, kind="ExternalInput").ap()
    bq = nc.dram_tensor("bq", [128, 2], f32, kind="ExternalInput").ap()
    bk = nc.dram_tensor("bk", [128, 2], f32, kind="ExternalInput").ap()
    out = nc.dram_tensor("out", [S, D], bf16, kind="ExternalOutput").ap()

    with tile.TileContext(nc) as tc:
        with (
            tc.tile_pool(name="wp", bufs=1) as wp,
            tc.tile_pool(name="xp", bufs=12) as xp,
            tc.tile_pool(name="qkv", bufs=1) as qkv,
            tc.tile_pool(name="cp", bufs=1) as cp,
            tc.tile_pool(name="ep", bufs=7) as ep,
            tc.tile_pool(name="rp", bufs=2) as rp,
            tc.tile_pool(name="op", bufs=2) as op,
            tc.tile_pool(name="pp", bufs=2, space="PSUM") as pp,
            tc.tile_pool(name="sp", bufs=2, space="PSUM") as sp,
            tc.tile_pool(name="cps", bufs=1, space="PSUM") as cps,
        ):
            # ---- constants first (no DMA deps) so warm-up can start now ----
            ones_sb = wp.tile([128, 64], bf16)
            nc.vector.memset(ones_sb, 1.0)
            warm_rhs = wp.tile([128, 512], bf16)
            nc.vector.memset(warm_rhs, 0.0)

            # V[tok, kt, head-of-4, 65]: col 64 of each head group stays at the
            # memset 1.0 -> PV row 64 is the softmax denominator.
            V = qkv.tile([128, KT_N, 4, 65], bf16)
            nc.vector.memset(V, 1.0)

            # warm-up matmuls: occupy the PE through the input-DMA wait so the
            # HAM clock gate reaches and KEEPS 8/8 until k_group(0,0) starts.
            for i in range(14):
                wps = pp.tile([64, 512], f32, tag="pp", name="wps")
                nc.tensor.matmul(wps, ones_sb[:, 0:64], warm_rhs)

            # ---- weights; DMA order = need order -------------------------
            wk_sb = wp.tile([128, 2, 8, 128], bf16)
            wq_sb = wp.tile([128, 2, 8, 128], bf16)
            wv_sb = wp.tile([128, 8, CL], bf16)
            wo_sb = wp.tile([128, 2, D], bf16)
            bq_sb = wp.tile([128, 2], f32)
            bk_sb = wp.tile([128, 2], f32)

            xk_t = [None] * TB
            xv_t = [None] * TB
            xq_t = [None] * QB

            def dma_x(kind, idx):
                t = xp.tile([128, 8, TBS], bf16, tag="x", name=f"x{kind}{idx}")
                src = {"k": kt4, "v": vt4, "q": qt4}[kind]
                nc.sync.dma_start(t, src[idx])
                return t

            # The 16 DMA queues drain in dispatch order at ~0.4GB/ms marginal,
            # ~2.4us per MB; dispatch order IS the arrival order, so it must
            # exactly match the compute need order (v5's xq1-at-slot-2 miss
            # stalled the PE FIFO 5.7us).  Everything is dispatched upfront
            # (xp has 12 bufs - no reuse, no item-DMA deadlocks).
            nc.sync.dma_start(wk_sb[:, 0], wkT[0])
            nc.sync.dma_start(bk_sb, bk)
            xk_t[0] = dma_x("k", 0)
            nc.sync.dma_start(wq_sb[:, 0], wqT[0])
            xq_t[0] = dma_x("q", 0)
            nc.sync.dma_start(bq_sb, bq)
            nc.sync.dma_start(wv_sb, wvT)
            xk_t[1] = dma_x("k", 1)
            xv_t[0] = dma_x("v", 0)
            xk_t[2] = dma_x("k", 2)
            xv_t[1] = dma_x("v", 1)
            xk_t[3] = dma_x("k", 3)
            xq_t[1] = dma_x("q", 1)
            xv_t[2] = dma_x("v", 2)
            xv_t[3] = dma_x("v", 3)
            nc.sync.dma_start(wk_sb[:, 1], wkT[1])
            xq_t[2] = dma_x("q", 2)
            nc.sync.dma_start(wq_sb[:, 1], wqT[1])
            xq_t[3] = dma_x("q", 3)
            nc.sync.dma_start(wo_sb, woT)

            # ---- activations in SBUF ----
            QT = qkv.tile([128, 2, S], bf16)   # [chan, tok]
            KT = qkv.tile([128, 2, S], bf16)

            # ---- projection group emitters ----
            def _proj_group(w_sb, x_t, b_sb, dst, tb, cc, half=None):
                t0 = tb * TBS
                lo, hi = (0, 8) if half is None else ((0, 4) if half == 0
                                                      else (4, 8))
                if lo == 0:
                    ps = pp.tile([128, TBS], f32, tag="pp", name="ps_p")
                    _proj_ps[(id(w_sb), tb, cc)] = ps
                else:
                    ps = _proj_ps.pop((id(w_sb), tb, cc))
                for c in range(lo, hi):
                    nc.tensor.matmul(ps, w_sb[:, cc, c], x_t[tb][:, c],
                                     start=(c == 0), stop=(c == 7))
                if hi == 8:
                    nc.vector.tensor_scalar_add(dst[:, cc, t0:t0 + TBS], ps,
                                                b_sb[:, cc:cc + 1])

            _proj_ps = {}

            def k_group(tb, cc, half=None):
                _proj_group(wk_sb, xk_t, bk_sb, KT, tb, cc, half)

            def q_group(qb, cc, half=None):
                _proj_group(wq_sb, xq_t, bq_sb, QT, qb, cc, half)

            def v_group(tb, tt):
                ps = pp.tile([128, CL], f32, tag="pp", name="ps_v")
                for c in range(8):
                    nc.tensor.matmul(ps, xv_t[tb][:, c, tt * 128:(tt + 1) * 128],
                                     wv_sb[:, c], start=(c == 0), stop=(c == 7))
                T = tb * 4 + tt
                nc.vector.tensor_copy(
                    V[:, T, :, 0:64],
                    ps.rearrange("p (g c) -> p g c", c=64))

            # ---- prologue projections: just enough for unit (0,0) kt0.
            # Junk matmuls bridge the xq0 DMA wait after k_group so the HAM
            # clock gate stays at 8/8 (an idle >3.4us re-throttles the PE and
            # q_group would run at 1.2GHz).
            k_group(0, 0)
            for i in range(6):
                wps = pp.tile([64, 512], f32, tag="pp", name="wps_b")
                nc.tensor.matmul(wps, ones_sb[:, 0:64], warm_rhs)
            q_group(0, 0)

            # ---- attention ----
            C = cp.tile([128, 2, S], bf16)   # C^T [cat-chan, tok]

            def make_norm(qb, hp, c):
                q0 = qb * QBS

                def norm():
                    # c is [65, 2, QBS]; row 64 = denominators for both heads
                    s_ab = rp.tile([65, 2, QBS], bf16, tag="sab", name="s_ab")
                    nc.vector.tensor_copy(s_ab[64:65], c[64:65])
                    b_ps = pp.tile([64, QBS], f32, tag="pp", name="b_ps")
                    nc.tensor.matmul(b_ps, ones_sb[64:65, :], s_ab[64:65, 0])
                    r_a = rp.tile([64, QBS], f32, tag="ra", name="r_a")
                    nc.vector.reciprocal_approx_fast(r_a, b_ps)
                    b_ps2 = pp.tile([64, QBS], f32, tag="pp", name="b_ps2")
                    nc.tensor.matmul(b_ps2, ones_sb[64:65, :], s_ab[64:65, 1])
                    r_b = rp.tile([64, QBS], f32, tag="rb", name="r_b")
                    nc.vector.reciprocal_approx_fast(r_b, b_ps2)
                    nc.vector.tensor_mul(C[0:64, hp, q0:q0 + QBS],
                                         c[0:64, 0], r_a)
                    nc.vector.tensor_mul(C[64:128, hp, q0:q0 + QBS],
                                         c[0:64, 1], r_b)
                return norm

            def make_outproj(qb, tt, act_copy=False):
                def outproj():
                    tg = qb * QBS + tt * 128
                    o = op.tile([128, D], bf16, tag="o", name="o")
                    ps0 = pp.tile([128, 512], f32, tag="pp", name="ps0")
                    ps1 = pp.tile([128, 512], f32, tag="pp", name="ps1")
                    for cc in range(2):
                        nc.tensor.matmul(ps0, C[:, cc, tg:tg + 128],
                                         wo_sb[:, cc, 0:512],
                                         start=(cc == 0), stop=(cc == 1))
                        nc.tensor.matmul(ps1, C[:, cc, tg:tg + 128],
                                         wo_sb[:, cc, 512:1024],
                                         start=(cc == 0), stop=(cc == 1))
                    if act_copy:
                        nc.scalar.copy(o[:, 0:512], ps0)
                    else:
                        nc.vector.tensor_copy(o[:, 0:512], ps0)
                    nc.vector.tensor_copy(o[:, 512:1024], ps1)
                    nc.sync.dma_start(out[tg:tg + 128, :], o)
                return outproj

            def seq(*fns):
                def run():
                    for f in fns:
                        f()
                return run

            def mk(fn, *args):
                return lambda: fn(*args)

            # ---- per-unit interleave schedules ----
            # unit 0 carries all v_groups (V[T] is consumed by its PV(T)) plus
            # the cc=0 k-projections; slots are deadline-exact (see docstring).
            items = {u: [] for u in range(len(UNITS))}
            # pops: PV(j-4) at kt j (j=4..11); kt12 pops PV(8..10); kt13-15
            # pop PV(11..13); leftover items precede PV(14),PV(15).  Every
            # v_group(T) below is emitted at/before the kt whose pop reads it;
            # the DMA arrival order above matches these slots.  None = empty
            # slot (in units 1-7 heavy PE items are spaced ~3 kts apart so the
            # ACT stream, which can only run ~1 exp ahead via the 2 score
            # buffers, recovers between them).
            # scores(kt) is emitted at step kt-2 BEFORE that step's item, so
            # k_group(tb,0) must sit at step <= 4*tb-3.
            items[0] = [
                mk(k_group, 1, 0),                                # kt1 <=kt1
                mk(v_group, 0, 0),                                # kt2  V0<=kt4
                mk(v_group, 0, 1),                                # kt3  V1<=kt5
                mk(v_group, 0, 2),                                # kt4  V2<=kt6
                mk(k_group, 2, 0),                                # kt5 <=kt5
                mk(v_group, 0, 3),                                # kt6  V3<=kt7
                mk(v_group, 1, 0),                                # kt7  V4<=kt8
                mk(v_group, 1, 1),                                # kt8  V5<=kt9
                seq(mk(k_group, 3, 0), mk(v_group, 1, 2)),        # kt9  V6<=kt10
                mk(v_group, 1, 3),                                # kt10 V7<=kt11
                mk(v_group, 2, 0),                                # kt11 V8<=kt12
                seq(mk(v_group, 2, 1), mk(v_group, 2, 2)),        # kt12 V9,V10
                mk(v_group, 2, 3),                                # kt13 V11<=kt13
                mk(v_group, 3, 0),                                # kt14 V12<=kt14
                seq(mk(q_group, 1, 0), mk(v_group, 3, 1)),        # kt15 V13<=kt15
                mk(v_group, 3, 2),                                # leftover V14
                mk(v_group, 3, 3),                                # leftover V15
            ]
            # k/q groups in units 1-7 are emitted as two 4-matmul halves in
            # adjacent slots: each half (~0.9us PE) fits inside one exp window
            # (~1.1us) so the ACT stream never starves (a full 8-MM group
            # overflowed by ~0.6us and cost ~1us of ACT idle per item).
            def halves(fn, a, b):
                return [mk(fn, a, b, 0), mk(fn, a, b, 1)]

            items[1] = (halves(k_group, 0, 1) + [None]
                        + halves(q_group, 2, 0))
            items[2] = (halves(q_group, 3, 0) + [None]
                        + halves(k_group, 1, 1))
            items[3] = (halves(q_group, 0, 1) + [None]
                        + halves(k_group, 2, 1))
            items[4] = (halves(q_group, 1, 1) + [None]
                        + halves(k_group, 3, 1))
            # outprojs start once both norms of a qb are done (norm(qb,1) of
            # unit 4+i is the first item of unit 5+i)
            items[5] = ([make_outproj(0, 0), None, make_outproj(0, 1), None]
                        + halves(q_group, 2, 1))
            items[6] = ([make_outproj(0, 2), None, make_outproj(0, 3), None,
                         make_outproj(1, 0), None] + halves(q_group, 3, 1))
            items[7] = [make_outproj(1, 1), None, make_outproj(1, 2), None,
                        make_outproj(1, 3), None, make_outproj(2, 0), None,
                        make_outproj(2, 1)]

            def pv(kt, e, c, hp):
                # C^T accumulation; row 64 = softmax denominators
                nc.tensor.matmul(c[:, 0], V[:, kt, 2 * hp],
                                 e[:, 0:QBS], start=(kt == 0),
                                 stop=(kt == KT_N - 1))
                nc.tensor.matmul(c[:, 1], V[:, kt, 2 * hp + 1],
                                 e[:, QBS:2 * QBS], start=(kt == 0),
                                 stop=(kt == KT_N - 1))

            last_es = []

            def attention_unit(ui, work, finish_prev=None, keep_es=False):
                # Software-pipelined: scores(kt+2) is emitted right after
                # exp(kt) and BEFORE the kt's work item, so the ACT stream has
                # a 2-exp runway (the 2 score buffers allow it; v5.1's 1-ahead
                # emission made every work item and every DVE-offloaded exp an
                # ACT bubble).  The previous unit's PV drain is emitted after
                # this unit's first scores+exp (finish_prev) for the same
                # reason.
                qb, hp = UNITS[ui]
                q0 = qb * QBS
                c = cps.tile([65, 2, QBS], f32, tag="c", name="c")
                dve_kts = DVE_EXP.get(ui, ())

                def scores(kt):
                    k0 = kt * 128
                    s_ps = sp.tile([128, 2 * QBS], f32, tag="s")
                    nc.tensor.matmul(s_ps[:, 0:QBS],
                                     KT[0:64, hp, k0:k0 + 128],
                                     QT[0:64, hp, q0:q0 + QBS])
                    nc.tensor.matmul(s_ps[:, QBS:2 * QBS],
                                     KT[64:128, hp, k0:k0 + 128],
                                     QT[64:128, hp, q0:q0 + QBS])
                    return s_ps

                s_tile = {}
                pend = []
                it = 0
                for kt in range(KT_N):
                    if kt == 0:
                        s_tile[0] = scores(0)
                        s_tile[1] = scores(1)
                    e = ep.tile([128, 2 * QBS], bf16, tag="e")
                    if kt in dve_kts:
                        nc.vector.tensor_scalar(e.bitcast(i16), s_tile.pop(kt),
                                                EXP_A, EXP_B, ALU.mult, ALU.add)
                    else:
                        nc.scalar.activation(e, s_tile.pop(kt), AF.Exp,
                                             scale=0.125)
                    if kt == 0 and finish_prev is not None:
                        finish_prev()
                    if kt + 2 < KT_N:
                        s_tile[kt + 2] = scores(kt + 2)
                    if keep_es and kt >= 13:
                        last_es.append(e)
                    pend.append((kt, e))
                    if kt >= 1 and it < len(work):
                        if work[it] is not None:
                            work[it]()
                        it += 1
                    # drain 2/kt from kt12 so the end-of-unit PV tail is short
                    npend = 4 if kt < 12 else 2
                    while len(pend) > npend:
                        pv(*pend.pop(0), c, hp)

                def finisher():
                    j = it
                    while j < len(work):
                        if work[j] is not None:
                            work[j]()
                        j += 1
                    for item in pend:
                        pv(*item, c, hp)
                return c, finisher

            prev_c = {}
            fin = None
            for ui in range(len(UNITS)):
                work = list(items[ui])
                if ui >= 1:
                    pu = UNITS[ui - 1]
                    work.insert(0, make_norm(pu[0], pu[1], prev_c[ui - 1]))
                prev_c[ui], fin = attention_unit(
                    ui, work, finish_prev=fin,
                    keep_es=(ui == len(UNITS) - 1))
            fin()

            # ---- tail: outproj(2,2..3) runs during the chunked norm(3,1);
            # outproj(3,t) follows chunk t; copies split ACT+DVE ------------
            lc = prev_c[len(UNITS) - 1]
            make_outproj(2, 2, act_copy=True)()
            make_outproj(2, 3, act_copy=True)()
            q0 = 3 * QBS
            for t in range(4):
                g0 = t * 128
                s_ab = rp.tile([65, 2, 128], bf16, tag="sabt", name="s_abt")
                nc.vector.tensor_copy(s_ab[64:65], lc[64:65, :, g0:g0 + 128])
                b_ps = pp.tile([64, 128], f32, tag="pp", name="tb_ps")
                nc.tensor.matmul(b_ps, ones_sb[64:65, :], s_ab[64:65, 0])
                r_a = rp.tile([64, 128], f32, tag="rat", name="r_at")
                nc.vector.reciprocal_approx_fast(r_a, b_ps)
                b_ps2 = pp.tile([64, 128], f32, tag="pp", name="tb_ps2")
                nc.tensor.matmul(b_ps2, ones_sb[64:65, :], s_ab[64:65, 1])
                r_b = rp.tile([64, 128], f32, tag="rbt", name="r_bt")
                nc.vector.reciprocal_approx_fast(r_b, b_ps2)
                nc.vector.tensor_mul(C[0:64, 1, q0 + g0:q0 + g0 + 128],
                                     lc[0:64, 0, g0:g0 + 128], r_a)
                nc.vector.tensor_mul(C[64:128, 1, q0 + g0:q0 + g0 + 128],
                                     lc[0:64, 1, g0:g0 + 128], r_b)
                make_outproj(3, t, act_copy=True)()

            # HAM keep-warm junk through the final exp drain (reads the last
            # unit's e tiles, whose ep buffers are never rotated again)
            for e in last_es:
                for h in range(2):
                    wps = pp.tile([64, 512], f32, tag="pp", name="wps_t")
                    nc.tensor.matmul(wps, ones_sb[:, 0:64], e[:, h * 512:h * 512 + 512])

    nc.compile()
    return nc


_CACHE = {}


def _get_nc():
    if "nc" not in _CACHE:
        _CACHE["nc"] = build()
    return _CACHE["nc"]


def make_in_maps(q, k, v, wq, bq, wk, bk, wv, bv, wo, bo):
    def tile4(x):
        # [S, D] activation -> [TB, 128, 8, TBS]; [tb,p,c,t] = x[tb*512+t, c*128+p]
        xT = np.ascontiguousarray(np.asarray(x).T).astype(BF)  # [D, S]
        return np.ascontiguousarray(
            xT.reshape(8, 128, TB, TBS).transpose(2, 1, 0, 3))

    def wtile_cc(w, sl):
        # [D_out slice, D_in] -> [2, 128, 8, 128]; [cc,p,c,j] = w.T[c*128+p, cc*128+j]
        wT = np.ascontiguousarray(np.asarray(w)[sl, :].T).astype(BF)  # [D, CL]
        t = wT.reshape(8, 128, 2, 128).transpose(2, 1, 0, 3)
        return np.ascontiguousarray(t)

    def wvtile(w, sl):
        wT = np.ascontiguousarray(np.asarray(w)[sl, :].T).astype(BF)  # [D, CL]
        return np.ascontiguousarray(wT.reshape(8, 128, CL).transpose(1, 0, 2))

    def wotile(wo_, sl):
        woTc = np.ascontiguousarray(np.asarray(wo_)[:, sl].T).astype(BF)  # [CL, D]
        return np.ascontiguousarray(woTc.reshape(2, 128, D).transpose(1, 0, 2))

    def btile(b, sl):
        return np.ascontiguousarray(
            np.asarray(b)[sl].astype(np.float32).reshape(2, 128).T)

    xt = {}
    for b in range(B):
        xt[("q", b)] = tile4(q[b])
        xt[("k", b)] = tile4(k[b])
        xt[("v", b)] = tile4(v[b])
    in_maps = []
    for core in range(N_CORES):
        b, j = divmod(core, N_CORES // B)
        sl = slice(CL * j, CL * (j + 1))
        in_maps.append({
            "qt4": xt[("q", b)],
            "kt4": xt[("k", b)],
            "vt4": xt[("v", b)],
            "wqT": wtile_cc(wq, sl),
            "wkT": wtile_cc(wk, sl),
            "wvT": wvtile(wv, sl),
            "woT": wotile(wo, sl),
            "bq": btile(bq, sl),
            "bk": btile(bk, sl),
        })
    return in_maps


def combine(results, bv, wo, bo):
    GP = N_CORES // B
    const = (np.asarray(bv, dtype=np.float64) @ np.asarray(wo, dtype=np.float64).T
             + np.asarray(bo, dtype=np.float64)).astype(np.float32)
    out = np.empty((B, S, D), dtype=np.float32)
    for b in range(B):
        acc = results[b * GP]["out"].astype(np.float32)
        for j in range(1, GP):
            acc = acc + results[b * GP + j]["out"].astype(np.float32)
        out[b] = acc + const[None, :]
    return out


def kernel(q, k, v, wq, bq, wk, bk, wv, bv, wo, bo):
    nc = _get_nc()
    in_maps = make_in_maps(q, k, v, wq, bq, wk, bk, wv, bv, wo, bo)
    res = bass_utils.run_bass_kernel_spmd(nc, in_maps, core_ids=list(range(N_CORES)))
    return combine(res.results, bv, wo, bo)
